# revision 31
# baseline (speedup 1.0000x reference)
"""Capsule-routing kernel for 8 TRN2 NeuronCores.

Strategy (n-sharded, u_hat never materialized):
  u_hat[b,n,c,d] = sum_i u[b,n,i] W[n,c,i,d] is only ever needed inside two
  contractions per routing iteration, both of which factor through W:
    (A) logits[b,n,c] = sum_d u_hat . Vacc  = sum_i u[b,n,i] * WV[b,n,c,i]
        with WV[b,n,c,i] = sum_d W[n,c,i,d] Vacc[b,c,d]   (PE matmul, p=d,
        bf16, out packed [(g4,b32); nl,i] for full-width DVE work)
    (B) s[b,c,d] = sum_n coup . u_hat = sum_{n,i} (coup[b,n,c] u[b,n,i]) W[n,c,i,d]
        p = (n16,i8) packed chunks, m = (c,d) halves of 80, f = (c',b) = 320,
        keeping only the diagonal c==c' blocks (all bf16).
  coup lives as [nl128; c,g,b] after a PE transpose; a constant replication
  matmul (R_w) re-packs it to [(n16,i); c,b] per 16-capsule chunk for (B).
  Each core owns 512 of the 4096 input capsules. Per-round global sums are
  AllGathered (20KB) and reduced on-chip; final reduce + squash on host.

v2 performance structure:
  - Round-0-critical loads (up_b, wp_b) go first on the SP DMA queue; the
    remaining constants load via the Activation HWDGE queue so the SP rings
    are clear for the round-0 partial write that gates AllGather #1.
  - All matmuls are bf16 (f32 matmuls are ~4x slower on the PE).
  - Squash scalar math runs transposed on [128, 3] tiles (a [1,320] layout
    uses one DVE lane; reciprocal there costs 2.1us).

Layouts (host-prepared; partition dim first):
  WA [128=(g4,d32) ; c10, nl128, i8]  d zero-padded 16->32, bf16
  WP [128=(n16,i8) ; ch32, c10, d16]  packed W, chunk = 16 capsules, bf16
  UP [128=(n16,i8) ; ch32, b32]       bf16
  u4 [128=(g4,b32) ; nl128, i8]       bf16
  RW [128=nl       ; w8, m128]  R_w[nl, (n16,i)] = (nl == w*16+n16), bf16
  isid [128=(k8,d16); d16]      identity stack for the 8-way gathered reduce
  rep16 [16=d      ; g4, d32]   replication matrix for vrep update
"""

import sys
import ml_dtypes
import numpy as np

sys.path.insert(0, "/opt/trn_rl_repo")

from contextlib import ExitStack

import concourse.bass as bass
import concourse.tile as tile
from concourse import bacc, mybir, masks
from concourse.bass_utils import run_bass_kernel_spmd

F32 = mybir.dt.float32
F32R = mybir.dt.float32r
BF16 = mybir.dt.bfloat16
AX = mybir.AxisListType
ALU = mybir.AluOpType
ACTF = mybir.ActivationFunctionType

B, N, C, DI, DV = 32, 4096, 10, 8, 16
NCORES = 8
NL = N // NCORES          # 512 capsules per core
G = 4                     # n-groups per core
NLG = NL // G             # 128 capsules per group
NCH = NL * DI // 128      # 32 packed (n16,i) chunks
NUM_ROUTING = 3
EPS = 1e-7


def _r(ap):
    return ap.bitcast(F32R)


def _body(ctx, tc, dins, out_d):
    nc = tc.nc

    consts = ctx.enter_context(tc.tile_pool(name="consts", bufs=1))
    persist = ctx.enter_context(tc.tile_pool(name="persist", bufs=1))
    work = ctx.enter_context(tc.tile_pool(name="work", bufs=2))
    psum_big = ctx.enter_context(tc.tile_pool(name="psum_big", bufs=2, space="PSUM"))
    psum_sm = ctx.enter_context(tc.tile_pool(name="psum_sm", bufs=2, space="PSUM"))
    dram = ctx.enter_context(tc.tile_pool(name="dram", bufs=1, space="DRAM"))

    # ---- constant / input loads ------------------------------------------
    wa = consts.tile([128, C, NLG, DI], BF16)
    wp_t = consts.tile([128, NCH, C, DV], BF16)
    up_t = consts.tile([128, NCH, B], BF16)
    u4 = consts.tile([G * B, NLG, DI], BF16)
    rw = consts.tile([NLG, 8, 128], BF16)
    isid = consts.tile([128, DV], BF16)
    rep16 = consts.tile([DV, G * 32], BF16)
    ident = consts.tile([128, 128], F32)
    identb = consts.tile([128, 128], BF16)

    # round-0 critical loads on the SP queue first: two big DMAs with
    # 10KB-per-partition descriptors (descriptor count, not bytes, bounds
    # the load phase)
    nc.sync.dma_start(wp_t[:], dins["wp_b"].ap().rearrange(
        "p (ch c d) -> p ch c d", ch=NCH, c=C))
    nc.scalar.dma_start(up_t[:], dins["up_b"].ap().rearrange(
        "p (ch b) -> p ch b", ch=NCH))

    def wp_b(ch):
        return wp_t[:, ch, :, :]

    def up_b(ch):
        return up_t[:, ch, :]
    masks.make_identity(nc, ident[:])
    masks.make_identity(nc, identb[:])

    def deferred_loads():
        # issued on the Activation HWDGE queue after round-0's PSUM copies,
        # so the critical wp/up prefix owns the DMA rings at startup; these
        # stream during the AllGather-1 window instead.
        nc.scalar.dma_start(wa[:], dins["wa"].ap().rearrange(
            "p (c nl i) -> p c nl i", c=C, nl=NLG))
        nc.scalar.dma_start(u4[:], dins["u4"].ap().rearrange(
            "p (nl i) -> p nl i", nl=NLG))
        nc.scalar.dma_start(rw[:], dins["rw"].ap().rearrange(
            "p (w m) -> p w m", w=8))
        nc.scalar.dma_start(isid[:], dins["isid"].ap())
        nc.scalar.dma_start(rep16[:], dins["rep16"].ap())

    # constant APs for activation bias operands
    czero = consts.tile([128, 1], F32)
    nc.gpsimd.memset(czero[:], 0.0)
    nc.const_aps.aps[(F32, 0.0)] = czero[:]
    ceps = consts.tile([128, 1], F32)
    nc.gpsimd.memset(ceps[:], EPS)
    nc.const_aps.aps[(F32, EPS)] = ceps[:]

    # persistent accumulators
    vrep = persist.tile([128, C, B], F32)       # [(g,d32); c,b] Vacc replicated
    vrep_r = persist.tile([128, C, B], BF16)    # bf16 copy for the PE (A) pass
    nc.gpsimd.memset(vrep[:], 0.0)

    coup_t = persist.tile([NLG, C, G, B], BF16)  # [nl; c, g, b]
    cup = persist.tile([128, NCH, C, B], BF16)   # [(n16,i); ch, c, b] packed cu
    logits = persist.tile([G * B, NLG, C], F32)

    def squash_update(ps_tot, last=False):
        """squash scale from PSUM stot [16; (c,b)], fold scale*stot into vrep.

        The 320 (c,b) scalars are transposed onto 128 partitions so the
        divide/sqrt run wide instead of on one DVE lane.
        """
        stot_sb = work.tile([DV, C * B], F32, tag="stot_sb")
        nc.vector.tensor_copy(stot_sb[:], ps_tot[:])
        # transpose chunks of 128 (c,b) columns -> [128, 3, 16]
        ps_sT = psum_sm.tile([128, 3, DV], F32, tag="ps_misc")
        for k in range(3):
            w = 128 if k < 2 else 64
            nc.tensor.transpose(
                ps_sT[0:w, k, :], stot_sb[:, 128 * k:128 * k + w], ident[0:DV, 0:DV])
        stT = work.tile([128, 3, DV], F32, tag="stT")
        nc.vector.tensor_copy(stT[:], ps_sT[:])
        sqT = work.tile([128, 3, DV], F32, tag="sqT")
        nc.vector.tensor_tensor(out=sqT[:], in0=stT[:], in1=stT[:], op=ALU.mult)
        s2T = work.tile([128, 3], F32, tag="s2T")
        nc.vector.tensor_reduce(s2T[:], sqT[:], axis=AX.X, op=ALU.add)
        qT = work.tile([128, 3], F32, tag="qT")
        nc.scalar.activation(qT[:], s2T[:], ACTF.Sqrt, bias=EPS)
        den = work.tile([128, 3], F32, tag="den3")
        nc.vector.scalar_tensor_tensor(out=den[:], in0=s2T[:], scalar=1.0,
                                       in1=qT[:], op0=ALU.add, op1=ALU.mult)
        rcp = work.tile([128, 3], F32, tag="rcp")
        nc.vector.reciprocal(rcp[:], den[:])
        scT = work.tile([128, 3], F32, tag="scT")
        nc.vector.tensor_tensor(out=scT[:], in0=s2T[:], in1=rcp[:], op=ALU.mult)
        # vsmallT = stotT * scale, then transpose back to [16, 320]
        vsT = work.tile([128, 3, DV], F32, tag="vsT")
        nc.vector.tensor_tensor(
            out=vsT[:], in0=stT[:],
            in1=scT[:].unsqueeze(2).broadcast_to((128, 3, DV)), op=ALU.mult)
        ps_vs = psum_sm.tile([DV, C * B], F32, tag="ps_misc")
        for k in range(3):
            w = 128 if k < 2 else 64
            nc.tensor.transpose(
                ps_vs[:, 128 * k:128 * k + w], vsT[0:w, k, :], ident[0:w, 0:w])
        vs_sb = work.tile([DV, C * B], BF16, tag="vs_sb")
        nc.vector.tensor_copy(vs_sb[:], ps_vs[:])
        ps_srep = psum_sm.tile([128, C * B], F32, tag="ps_misc")
        nc.tensor.matmul(ps_srep[:], lhsT=rep16[:], rhs=vs_sb[:],
                         start=True, stop=True)
        if last:
            nc.vector.tensor_tensor(
                out=vrep_r[:].rearrange("p c b -> p (c b)"),
                in0=vrep[:].rearrange("p c b -> p (c b)"),
                in1=ps_srep[:], op=ALU.add)
        else:
            nc.vector.tensor_tensor(
                out=vrep[:].rearrange("p c b -> p (c b)"),
                in0=vrep[:].rearrange("p c b -> p (c b)"),
                in1=ps_srep[:], op=ALU.add)
            nc.vector.tensor_copy(vrep_r[:].rearrange("p c b -> p (c b)"),
                                  vrep[:].rearrange("p c b -> p (c b)"))

    def reduce8(gath):
        """sum the 8 gathered partials [(k,d); (c,b)] -> PSUM stot [16; (c,b)]."""
        ps_tot = psum_sm.tile([DV, C * B], F32, tag="ps_misc")
        nc.tensor.matmul(ps_tot[:], lhsT=isid[:], rhs=gath[:],
                         start=True, stop=True)
        return ps_tot

    def gather(write_part, rnd):
        """AllGather the partial sum in [d; (c,b)] DRAM layout, reduce."""
        d_part = dram.tile([DV, C * B], BF16, tag=f"dpart{rnd}")
        d_gath = dram.tile([NCORES * DV, C * B], BF16, tag=f"dgath{rnd}")
        write_part(d_part)
        warm = work.tile([1, 1], F32, tag="warm")
        nc.scalar.activation(warm[:], czero[0:1, :], ACTF.Sqrt)
        nc.gpsimd.collective_compute(
            "AllGather", ALU.bypass, replica_groups=[list(range(NCORES))],
            ins=[d_part[:].opt()], outs=[d_gath[:].opt()])
        gath = work.tile([128, C * B], BF16, tag="gath")
        nc.sync.dma_start(gath[:], d_gath[:])
        return reduce8(gath[:])

    GROUPS = ((0, 8), (8, 2))   # (c_base, n_capsules) -> m = 128 / 32

    def bsum(rhs_for):
        """(B): psum groups [(c,d16); f], PSUM-accumulated over the 32
        packed chunks. Group 0 = capsules 0-7 (m=128), group 1 = 8-9."""
        groups = []
        fdim = rhs_for(0).free_size()
        for c0, nc_ in GROUPS:
            ph = psum_big.tile([16 * nc_, fdim], F32, tag=f"ps_b{c0}", bufs=1)
            for ch in range(NCH):
                nc.tensor.matmul(
                    ph[:],
                    lhsT=wp_b(ch)[:, c0:c0 + nc_, :].rearrange(
                        "p c d -> p (c d)"),
                    rhs=rhs_for(ch),
                    start=(ch == 0), stop=(ch == NCH - 1),
                    skip_group_check=True,
                )
            groups.append(ph)
        return groups

    def compact_write(groups, d_part, scale):
        """Copy bsum PSUM groups to SBUF bf16 (scaled), then DMA the
        diagonal [16,32] blocks straight into d_part [d, c, b]."""
        dp = d_part[:].rearrange("d (c b) -> d c b", c=C)
        for (c0, nc_), grp in zip(GROUPS, groups):
            fdim = grp[:].shape[-1]
            s_f = work.tile([16 * nc_, fdim], BF16, tag=f"s_fb{c0}")
            if scale == 1.0:
                nc.vector.tensor_copy(s_f[:], grp[:])
            else:
                nc.vector.tensor_scalar(s_f[:], grp[:], scale, None,
                                        op0=ALU.mult)
            diag = fdim == C * B
            for cl in range(nc_):
                c = c0 + cl
                src = (s_f[16 * cl:16 * (cl + 1), c * B:(c + 1) * B] if diag
                       else s_f[16 * cl:16 * (cl + 1), :])
                nc.sync.dma_start(dp[:, c, :], src)

    def round0():
        groups = bsum(lambda ch: up_b(ch))

        def write_part(d_part):
            compact_write(groups, d_part, 1.0 / C)
            deferred_loads()
        return gather(write_part, 0)

    def round12(rnd, last):
        # ---- (A): WV then logits -----------------------------------------
        for c in range(C):
            ps_wv = psum_big.tile([128, NLG, DI], F32, tag="ps_wv")
            for h in range(2):
                for g in range(G):
                    nc.tensor.matmul(
                        ps_wv[32 * g:32 * (g + 1),
                              64 * h:64 * (h + 1), :].rearrange(
                            "p nl i -> p (nl i)"),
                        lhsT=vrep_r[32 * g:32 * (g + 1), c, :],
                        rhs=wa[32 * g:32 * (g + 1), c,
                               64 * h:64 * (h + 1), :],
                        start=True, stop=True,
                        tile_position=(32 * g, 32 * g),
                    )
            wv_sb = work.tile([128, NLG, DI], BF16, tag="wv_sb")
            nc.scalar.copy(wv_sb[:], ps_wv[:])
            wvu = work.tile([128, NLG, DI], BF16, tag="wvu")
            nc.vector.tensor_tensor(out=wvu[:], in0=wv_sb[:], in1=u4[:],
                                    op=ALU.mult)
            nc.vector.tensor_reduce(
                logits[:, :, c], wvu[:], axis=AX.X, op=ALU.add)

        # ---- softmax over c ----------------------------------------------
        expd = work.tile([G * B, NLG, C], BF16, tag="expd")
        nc.scalar.activation(expd[:], logits[:], ACTF.Exp)
        den = work.tile([G * B, NLG], F32, tag="den")
        nc.vector.tensor_reduce(den[:], expd[:], axis=AX.X, op=ALU.add)
        rden = work.tile([G * B, NLG], F32, tag="rden")
        nc.vector.reciprocal(rden[:], den[:])
        rden_b = work.tile([G * B, NLG], BF16, tag="rden_b")
        nc.vector.tensor_copy(rden_b[:], rden[:])
        coupq = work.tile([G * B, NLG, C], BF16, tag="coupq")
        nc.vector.tensor_tensor(
            out=coupq[:], in0=expd[:],
            in1=rden_b[:].unsqueeze(2).broadcast_to((G * B, NLG, C)),
            op=ALU.mult)

        # ---- transpose coup to [nl; c,g,b] -------------------------------
        for c in range(C):
            ps_tr = psum_sm.tile([NLG, G * B], BF16, tag="ps_misc")
            nc.tensor.transpose(ps_tr[:], coupq[:, :, c], identb[:])
            nc.vector.tensor_copy(
                coup_t[:, c, :, :].rearrange("p g b -> p (g b)"), ps_tr[:])

        # ---- repack coup to [(n16,i); ch, c, b] and fold in u ------------
        # Even chunks: DVE multiplies straight out of PSUM (1x mode).
        # Odd chunks: Act copies PSUM->SBUF bf16, DVE multiplies at 2x.
        for ch in range(NCH):
            g, w = ch // 8, ch % 8
            ps_rep = psum_sm.tile([128, C, B], F32, tag="ps_misc")
            nc.tensor.matmul(
                ps_rep[:].rearrange("p c b -> p (c b)"),
                lhsT=rw[:, w, :],
                rhs=coup_t[:, :, g, :],
                start=True, stop=True,
            )
            if ch % 2 == 0:
                nc.vector.tensor_tensor(
                    out=cup[:, ch, :, :],
                    in0=ps_rep[:],
                    in1=up_b(ch).unsqueeze(1).broadcast_to(
                        (128, C, B)),
                    op=ALU.mult)
            else:
                rep_sb = work.tile([128, C, B], BF16, tag="rep_sb")
                nc.scalar.copy(rep_sb[:].rearrange("p c b -> p (c b)"),
                               ps_rep[:].rearrange("p c b -> p (c b)"))
                nc.vector.tensor_tensor(
                    out=cup[:, ch, :, :],
                    in0=rep_sb[:],
                    in1=up_b(ch).unsqueeze(1).broadcast_to(
                        (128, C, B)),
                    op=ALU.mult)

        # ---- (B) diagonal-waste weighted sum -----------------------------
        groups = bsum(lambda ch: cup[:, ch, :, :])

        if last:
            for gi, ((c0, nc_), grp) in enumerate(zip(GROUPS, groups)):
                s_f = work.tile([16 * nc_, C * B], F32, tag=f"s_f{c0}")
                nc.scalar.copy(s_f[:], grp[:])
                nc.sync.dma_start(out_d.ap()[128 * gi:128 * gi + 16 * nc_, :],
                                  s_f[:])
            return None

        def write_part(d_part):
            compact_write(groups, d_part, 1.0)
        return gather(write_part, rnd)

    stot = round0()
    squash_update(stot)
    stot = round12(1, last=False)
    squash_update(stot, last=True)
    round12(2, last=True)


IN_SHAPES = {
    "wa": [128, C * NLG * DI],
    "u4": [G * B, NLG * DI],
    "rw": [NLG, 8 * 128],
    "wp_b": [128, NCH * C * DV],
    "up_b": [128, NCH * B],
    "isid": [128, DV],
    "rep16": [DV, G * 32],
}

BF16_INS = ("wa", "u4", "wp_b", "up_b", "rw", "isid", "rep16")


def build_nc():
    nc = bacc.Bacc("TRN2", target_bir_lowering=False, debug=False,
                   num_devices=NCORES)
    dins = {name: nc.dram_tensor(name, shape,
                                 BF16 if name in BF16_INS else F32,
                                 kind="ExternalInput")
            for name, shape in IN_SHAPES.items()}
    # out keeps the [(c,d); b] flat layout: row = c*16+d
    out_d = nc.dram_tensor("out", [160, C * B], F32, kind="ExternalOutput")

    with tile.TileContext(nc) as tc, ExitStack() as ctx:
        _body(ctx, tc, dins, out_d)
    nc.compile()
    return nc


# --------------------------------------------------------------------------
# Host side
# --------------------------------------------------------------------------

def make_in_maps(x, W):
    x = np.ascontiguousarray(np.asarray(x, dtype=np.float32))
    W = np.ascontiguousarray(np.asarray(W, dtype=np.float32))
    u = x.reshape(B, N, DI)
    isid = np.tile(np.eye(DV, dtype=np.float32), (NCORES, 1)).astype(ml_dtypes.bfloat16)
    rep16 = np.zeros((DV, G, 32), np.float32)
    for d in range(DV):
        rep16[d, :, d] = 1.0
    rep16 = rep16.reshape(DV, G * 32).astype(ml_dtypes.bfloat16)
    rwm = np.zeros((NLG, 8, 128), np.float32)
    for w in range(8):
        for n16 in range(16):
            rwm[w * 16 + n16, w, n16 * DI:(n16 + 1) * DI] = 1.0
    rwm = rwm.reshape(NLG, 8 * 128).astype(ml_dtypes.bfloat16)

    in_maps = []
    for k in range(NCORES):
        sl = u[:, k * NL:(k + 1) * NL, :]                   # [B, 512, 8]
        Wk = W[k * NL:(k + 1) * NL]                         # [512, C, DI, DV]
        Wk_g = Wk.reshape(G, NLG, C, DI, DV)
        wa = np.zeros((G, 32, C, NLG, DI), np.float32)
        wa[:, :DV] = Wk_g.transpose(0, 4, 2, 1, 3)          # [g,d,c,nl,i]
        # packed: p = (n16, i), chunks of 16 n
        Wp = Wk.reshape(NCH, 16, C, DI, DV).transpose(1, 3, 0, 2, 4)
        # -> [n16, i, ch, c, d]
        Up = sl.reshape(B, NCH, 16, DI).transpose(2, 3, 1, 0)  # [n16,i,ch,b]
        u4 = sl.reshape(B, G, NLG, DI).transpose(1, 0, 2, 3)   # [g,b,nl,i]
        in_maps.append({
            "wa": np.ascontiguousarray(
                wa.reshape(128, C * NLG * DI)).astype(ml_dtypes.bfloat16),
            "wp_b": np.ascontiguousarray(
                Wp.reshape(128, NCH * C * DV)).astype(ml_dtypes.bfloat16),
            "up_b": np.ascontiguousarray(
                Up.reshape(128, NCH * B)).astype(ml_dtypes.bfloat16),
            "u4": np.ascontiguousarray(
                u4.reshape(G * B, NLG * DI)).astype(ml_dtypes.bfloat16),
            "rw": rwm,
            "isid": isid,
            "rep16": rep16,
        })
    return in_maps


def postprocess(outs):
    """outs: per core [160, C*B]: group0 rows (c8,d16), group1 rows (c2,d16);
    diagonal blocks [16d x 32b] at column c*B."""
    s = np.zeros((C, DV, B), np.float64)
    for o in outs:
        o = o.reshape(160, C, B)
        for c in range(8):
            s[c] += o[16 * c:16 * (c + 1), c, :].astype(np.float64)
        for c in range(8, C):
            r = 128 + 16 * (c - 8)
            s[c] += o[r:r + 16, c, :].astype(np.float64)
    s = s.transpose(2, 0, 1)                                # [b, c, d]
    s2 = np.sum(s * s, axis=-1, keepdims=True)
    v = (s2 / (1.0 + s2) / np.sqrt(s2 + EPS)) * s
    return v.astype(np.float32)


_NC_CACHE = {}


def kernel(x, W):
    if "nc" not in _NC_CACHE:
        _NC_CACHE["nc"] = build_nc()
    nc = _NC_CACHE["nc"]
    in_maps = make_in_maps(x, W)
    res = run_bass_kernel_spmd(nc, in_maps, list(range(NCORES)))
    outs = [res.results[k]["out"] for k in range(NCORES)]
    return postprocess(outs)
